# revision 15
# baseline (speedup 1.0000x reference)
"""CosineDistanceLoss kernel for Trainium2 (8 NeuronCores, Bass).

reference: mean_n(1 - sum_d feats[d,n] * warped_feats[d,n])
         = 1 - (1/N) * sum_{d,n} feats[d,n] * warped_feats[d,n]

The loss is a single global sum of the elementwise product, so ANY disjoint
partition of elements across cores is valid. The kernel is pure HBM streaming
(64 MiB/stack total; DVE has ~3x slack), and the measured per-core HBM
bandwidth is ASYMMETRIC and partly stable (nc0 sustains ~320 GB/s while its
stack partner nc1 gets ~401 GB/s; other cores land in 335-402). Since the
graded time is the MAX over cores, we balance: each core gets a slice of the
global element stream sized proportionally to its measured bandwidth.

Mechanics (one NEFF for all cores, shapes must match):
  - The global 2^25-element stream per tensor is cut into 256 chunks of
    128x1024 (0.5 MiB). Core i takes n_i consecutive chunks (sum n_i = 256),
    packed by the host into a [128, CAP*1024] DRAM buffer (first n_i*1024
    cols are real data, rest never read).
  - The kernel has CAP=36 chunk slots: HEAD_REAL unconditional slots, then
    COND_SLOTS conditional slots, then an unconditional tail. A core with
    nact active chunks skips the FIRST (CAP-nact) conditional slots
    (cond DMAs with bounds_check=skip_entire_dma: no data moved, semaphore
    still bumped), so the static DVE/sem pipeline is unchanged. The STT for
    a skipped slot reads garbage SBUF into an acc column the host ignores —
    and because skipped slots sit near the FRONT of the schedule, those
    stale STTs run during early-stream DVE slack instead of serializing
    after the last real chunk (which is always unconditional). nact is a
    per-core uint32 input pulled into a Sync register (~1us HBM ldr) between
    the slot-0 and slot-1 DMA issues, well before its first use.
  - Per chunk one fused DVE scalar_tensor_tensor (elementwise mult + free-
    axis add-reduce via accum_out; product discarded through a stride-0
    broadcast output) accumulates into acc[:, j]. Host combines the
    8 x [128, n_i] partials in float64.

Raw hand-rolled semaphores (no TileContext): avoids its ~7us preamble +
~10us epilogue. The NRT-injected postamble (all-sem zeroing, ~7us) and the
const-AP preamble (~2us) are fixed costs outside kernel control.
"""

import os

import numpy as np

import concourse.bacc as bacc
import concourse.mybir as mybir
from concourse.bass_utils import run_bass_kernel_spmd

D, N = 512, 65536
NCORES = 8
P = 128                          # SBUF partitions
TOTAL_ELEMS = D * N              # 2^25 per tensor

F = int(os.environ.get("COSLOSS_FCHUNK", "1024"))    # cols per chunk
CAP = int(os.environ.get("COSLOSS_CAP", "36"))       # chunk slots in the NEFF
HEAD_REAL = int(os.environ.get("COSLOSS_HEAD", "4"))  # unconditional head slots
COND_SLOTS = int(os.environ.get("COSLOSS_COND", "12"))  # conditional slots
MINCH = CAP - COND_SLOTS                             # minimum active chunks
SLOTS = int(os.environ.get("COSLOSS_SLOTS", "16"))   # SBUF ring slots
CHUNK_ELEMS = P * F                                  # 131072 for F=1024
GLOBAL_CHUNKS = TOTAL_ELEMS // CHUNK_ELEMS           # 256 for F=1024

# Per-device relative bandwidth weights (jax device order). Measured per-core
# HBM GB/s over repeated runs on this 8-core terminal; device 6 (physical
# nc0) is stably slow, device 7 (nc1) stably fast, the rest race in between.
_DEFAULT_WEIGHTS = (355.0, 386.0, 375.0, 402.0, 373.0, 394.0, 320.0, 401.0)

IMPL = os.environ.get("COSLOSS_IMPL", "bal")

_CACHE = {}


def _chunk_alloc(weights=None):
    """Largest-remainder apportionment of GLOBAL_CHUNKS to 8 cores."""
    if weights is None:
        env = os.environ.get("COSLOSS_N")
        if env:
            n = [int(x) for x in env.split(",")]
            assert len(n) == NCORES and sum(n) == GLOBAL_CHUNKS, n
            return n
        wenv = os.environ.get("COSLOSS_WEIGHTS")
        weights = (
            [float(x) for x in wenv.split(",")] if wenv else list(_DEFAULT_WEIGHTS)
        )
    w = np.asarray(weights, dtype=np.float64)
    exact = GLOBAL_CHUNKS * w / w.sum()
    n = np.floor(exact).astype(int)
    rem = exact - n
    for i in np.argsort(-rem)[: GLOBAL_CHUNKS - n.sum()]:
        n[i] += 1
    n = np.clip(n, MINCH + 1, CAP)
    # rebalance if clipping broke the sum (shift to/from the largest slots)
    while n.sum() != GLOBAL_CHUNKS:
        if n.sum() < GLOBAL_CHUNKS:
            i = np.argmin(n / w)
            assert n[i] < CAP
            n[i] += 1
        else:
            i = np.argmax(n / w)
            assert n[i] > MINCH + 1
            n[i] -= 1
    assert n.sum() == GLOBAL_CHUNKS and (n > MINCH).all() and (n <= CAP).all(), n
    return [int(x) for x in n]


def _build_balanced():
    import contextlib

    import concourse.bass as bass  # noqa: F401  (engine types via mybir)

    nc = bacc.Bacc(None)
    ncols = CAP * F
    f_in = nc.declare_dram_parameter("feats", [P, ncols], mybir.dt.float32, isOutput=False)
    w_in = nc.declare_dram_parameter("warped", [P, ncols], mybir.dt.float32, isOutput=False)
    nact_in = nc.declare_dram_parameter("nact", [1, 1], mybir.dt.uint32, isOutput=False)
    # CAP+1 acc/out cols: col CAP holds the second half-STT of the last slot.
    out = nc.declare_dram_parameter(
        "partial", [P, CAP + 1], mybir.dt.float32, isOutput=True
    )

    head = CAP - 4  # acc cols written out early vs at the end
    assert SLOTS <= MINCH and 2 * SLOTS * F * 4 <= 176 * 1024, (SLOTS, F)
    assert 2 <= HEAD_REAL and HEAD_REAL + COND_SLOTS <= CAP

    with (
        nc.sbuf_tensor([P, SLOTS * F], mybir.dt.float32) as ftile,
        nc.sbuf_tensor([P, SLOTS * F], mybir.dt.float32) as wtile,
        nc.sbuf_tensor([P, CAP + 1], mybir.dt.float32) as acc,
        nc.sbuf_tensor([P, 1], mybir.dt.float32) as dummy,
    ):
        with contextlib.ExitStack() as ctx:
            dsems = [ctx.enter_context(nc.semaphore(f"dsem{j}")) for j in range(CAP)]
            vsem = ctx.enter_context(nc.semaphore("vsem"))
            osem = ctx.enter_context(nc.semaphore("osem"))
            nact_reg = ctx.enter_context(nc.sync.register("nact_reg"))
            sem_nums = sorted(s.num for s in [*dsems, vsem, osem])
            assert sem_nums == list(
                range(sem_nums[0], sem_nums[0] + len(sem_nums))
            ), sem_nums
            sem_range = range(sem_nums[0], sem_nums[-1] + 1)

            with nc.Block(no_gpsimd_drain=True) as block:

                @block.sync
                def _(sync):
                    nact = None
                    for j in range(CAP):
                        if j == 1:
                            # Load this core's active-chunk count while the
                            # slot-0 data drains; first used at slot
                            # HEAD_REAL, long after the ~1us HBM ldr lands.
                            sync.reg_load(nact_reg, nact_in[0:1, 0:1])
                            nact = sync.snap(nact_reg, min_val=0, max_val=CAP)
                        s = j % SLOTS
                        if j >= SLOTS:
                            # WAR: slot s is being read by STT_{j-SLOTS};
                            # HWDGE issue is FIFO per ring, so this wait
                            # also orders this ring's later DMAs behind it.
                            sync.wait_ge(vsem, j - SLOTS + 1)
                        in_cond_block = HEAD_REAL <= j < HEAD_REAL + COND_SLOTS
                        # slot j in the cond block is skipped iff
                        # j - HEAD_REAL < CAP - nact  (first CAP-nact slots)
                        kw = (
                            {"cond": nact > CAP - j + HEAD_REAL - 1}
                            if in_cond_block
                            else {}
                        )
                        sync.dma_start(
                            ftile[:, s * F : (s + 1) * F],
                            f_in[:, j * F : (j + 1) * F],
                            **kw,
                        ).then_inc(dsems[j], 16)
                        sync.dma_start(
                            wtile[:, s * F : (s + 1) * F],
                            w_in[:, j * F : (j + 1) * F],
                            **kw,
                        ).then_inc(dsems[j], 16)
                    # Write out the bulk of acc early (overlaps the tail of
                    # the input stream); only the last columns stay on the
                    # post-stream critical path.
                    sync.wait_ge(vsem, head)
                    sync.dma_start(out[:, :head], acc[:, :head]).then_inc(osem, 16)
                    sync.wait_ge(vsem, CAP + 1)
                    sync.dma_start(out[:, head:], acc[:, head:]).then_inc(osem, 16)

                @block.vector
                def _(vector):
                    for j in range(CAP):
                        s = j % SLOTS
                        vector.wait_ge(dsems[j], 32)
                        # out = (ft * 1.0) * wt (discarded via stride-0
                        # broadcast), accum_out = per-partition sum. For a
                        # cond-skipped slot this reads garbage SBUF into an
                        # acc column the host masks out. The last slot is
                        # processed as two half-STTs (summed by the host) so
                        # only ~F/2 cols of DVE work trail the final DMA.
                        pieces = (
                            [(0, F)]
                            if j < CAP - 1
                            else [(0, F // 2), (F // 2, F - F // 2)]
                        )
                        for pi, (poff, psz) in enumerate(pieces):
                            col = j if pi == 0 else CAP
                            lo = s * F + poff
                            nc.vector.scalar_tensor_tensor(
                                dummy[:, :].broadcast_to((P, psz)),
                                ftile[:, lo : lo + psz],
                                1.0,
                                wtile[:, lo : lo + psz],
                                op0=mybir.AluOpType.mult,
                                op1=mybir.AluOpType.mult,
                                accum_out=acc[:, col : col + 1],
                            ).then_inc(vsem, 1)

                @block.gpsimd
                def _(gpsimd):
                    # osem at its final value implies both out-DMAs landed,
                    # which implies every earlier sem reached its final
                    # value. Reset them so the NEFF is safe to re-execute.
                    gpsimd.wait_ge(osem, 32)
                    gpsimd.dma_reset(sem_range)
                    gpsimd.sem_clear(sem_range)

    nc.finalize()
    return nc


def _build_raw_even():
    """Previous even-shard builder (64 rows/core, F=2048) kept as fallback."""
    nc = bacc.Bacc(None)
    import contextlib

    F0, M0, SLOTS0 = 2048, 32768, 8
    nch = M0 // F0  # 16
    chunks = []
    for j in range(nch):
        if j == nch - 1:
            q = F0 // 4
            for k in range(4):
                chunks.append((j * F0 + k * q, q))
        else:
            chunks.append((j * F0, F0))
    nchunks = len(chunks)
    head = nchunks - 4
    f_in = nc.declare_dram_parameter("feats", [P, M0], mybir.dt.float32, isOutput=False)
    w_in = nc.declare_dram_parameter("warped", [P, M0], mybir.dt.float32, isOutput=False)
    out = nc.declare_dram_parameter("partial", [P, nchunks], mybir.dt.float32, isOutput=True)

    with (
        nc.sbuf_tensor([P, SLOTS0 * F0], mybir.dt.float32) as ftile,
        nc.sbuf_tensor([P, SLOTS0 * F0], mybir.dt.float32) as wtile,
        nc.sbuf_tensor([P, nchunks], mybir.dt.float32) as acc,
        nc.sbuf_tensor([P, 1], mybir.dt.float32) as dummy,
    ):
        with contextlib.ExitStack() as ctx:
            dsems = [ctx.enter_context(nc.semaphore(f"dsem{j}")) for j in range(nchunks)]
            vsem = ctx.enter_context(nc.semaphore("vsem"))
            osem = ctx.enter_context(nc.semaphore("osem"))
            sem_nums = sorted(s.num for s in [*dsems, vsem, osem])
            assert sem_nums == list(range(sem_nums[0], sem_nums[0] + len(sem_nums)))
            sem_range = range(sem_nums[0], sem_nums[-1] + 1)

            with nc.Block(no_gpsimd_drain=True) as block:

                @block.sync
                def _(sync):
                    for j, (off, sz) in enumerate(chunks):
                        s = j % SLOTS0
                        if j >= SLOTS0:
                            sync.wait_ge(vsem, j - SLOTS0 + 1)
                        sync.dma_start(
                            ftile[:, s * F0 : s * F0 + sz], f_in[:, off : off + sz]
                        ).then_inc(dsems[j], 16)
                        sync.dma_start(
                            wtile[:, s * F0 : s * F0 + sz], w_in[:, off : off + sz]
                        ).then_inc(dsems[j], 16)
                    sync.wait_ge(vsem, head)
                    sync.dma_start(out[:, :head], acc[:, :head]).then_inc(osem, 16)
                    sync.wait_ge(vsem, nchunks)
                    sync.dma_start(out[:, head:], acc[:, head:]).then_inc(osem, 16)

                @block.vector
                def _(vector):
                    for j, (off, sz) in enumerate(chunks):
                        s = j % SLOTS0
                        vector.wait_ge(dsems[j], 32)
                        nc.vector.scalar_tensor_tensor(
                            dummy[:, :].broadcast_to((P, sz)),
                            ftile[:, s * F0 : s * F0 + sz],
                            1.0,
                            wtile[:, s * F0 : s * F0 + sz],
                            op0=mybir.AluOpType.mult,
                            op1=mybir.AluOpType.mult,
                            accum_out=acc[:, j : j + 1],
                        ).then_inc(vsem, 1)

                @block.gpsimd
                def _(gpsimd):
                    gpsimd.wait_ge(osem, 32)
                    gpsimd.dma_reset(sem_range)
                    gpsimd.sem_clear(sem_range)

    nc.finalize()
    return nc


def _get_nc(impl=None):
    impl = impl or IMPL
    if impl not in _CACHE:
        _CACHE[impl] = _build_balanced() if impl == "bal" else _build_raw_even()
    return _CACHE[impl]


def _active_slots(n_chunks):
    m = CAP - n_chunks  # skipped = first m slots of the cond block
    assert 0 <= m <= COND_SLOTS, n_chunks
    return list(range(HEAD_REAL)) + list(range(HEAD_REAL + m, CAP))


def _active_cols(n_chunks):
    # acc col CAP is the second half of (always-real) slot CAP-1.
    return _active_slots(n_chunks) + [CAP]


def _pack(flat, start_chunk, n_chunks):
    buf = np.zeros((P, CAP * F), dtype=np.float32)
    sl = flat[
        start_chunk * CHUNK_ELEMS : (start_chunk + n_chunks) * CHUNK_ELEMS
    ].reshape(n_chunks, P, F)
    for i, slot in enumerate(_active_slots(n_chunks)):
        buf[:, slot * F : (slot + 1) * F] = sl[i]
    return buf


def _run(feats, warped_feats, impl=None, **spmd_kwargs):
    feats = np.ascontiguousarray(np.asarray(feats), dtype=np.float32)
    warped = np.ascontiguousarray(np.asarray(warped_feats), dtype=np.float32)
    assert feats.shape == (D, N) and warped.shape == (D, N)
    impl = impl or IMPL

    if impl == "bal":
        n = _chunk_alloc()
        ff, wf = feats.reshape(-1), warped.reshape(-1)
        starts = np.concatenate([[0], np.cumsum(n)])
        in_maps = [
            {
                "feats": _pack(ff, starts[c], n[c]),
                "warped": _pack(wf, starts[c], n[c]),
                "nact": np.array([[n[c]]], dtype=np.uint32),
            }
            for c in range(NCORES)
        ]
    else:
        n = None
        DSHARD, M0 = D // NCORES, 32768
        in_maps = [
            {
                "feats": feats[c * DSHARD : (c + 1) * DSHARD].reshape(P, M0),
                "warped": warped[c * DSHARD : (c + 1) * DSHARD].reshape(P, M0),
            }
            for c in range(NCORES)
        ]
    res = run_bass_kernel_spmd(
        _get_nc(impl), in_maps, core_ids=list(range(NCORES)), **spmd_kwargs
    )
    res.chunk_alloc = n
    return res


def gather_partials(res):
    """Mask-aware reduction of per-core partials to the scalar loss."""
    n = getattr(res, "chunk_alloc", None)
    total = 0.0
    for c, r in enumerate(res.results):
        p = r["partial"].astype(np.float64)
        if n is not None:
            p = p[:, _active_cols(n[c])]
        total += float(p.sum())
    return np.array(1.0 - total / N, dtype=np.float32)


def kernel(feats, warped_feats):
    return gather_partials(_run(feats, warped_feats))


# revision 16
# speedup vs baseline: 1.0166x; 1.0166x over previous
"""CosineDistanceLoss kernel for Trainium2 (8 NeuronCores, Bass).

reference: mean_n(1 - sum_d feats[d,n] * warped_feats[d,n])
         = 1 - (1/N) * sum_{d,n} feats[d,n] * warped_feats[d,n]

The loss is a single global sum of the elementwise product, so ANY disjoint
partition of elements across cores is valid. The kernel is pure HBM streaming
(64 MiB/stack total; DVE has ~3x slack), and the measured per-core HBM
bandwidth is ASYMMETRIC and partly stable (nc0 sustains ~320 GB/s while its
stack partner nc1 gets ~401 GB/s; other cores land in 335-402). Since the
graded time is the MAX over cores, we balance: each core gets a slice of the
global element stream sized proportionally to its measured bandwidth.

Mechanics (one NEFF for all cores, shapes must match):
  - The global 2^25-element stream per tensor is cut into 256 chunks of
    128x1024 (0.5 MiB). Core i takes n_i consecutive chunks (sum n_i = 256),
    packed by the host into a [128, CAP*1024] DRAM buffer (first n_i*1024
    cols are real data, rest never read).
  - The kernel has CAP=36 chunk slots: HEAD_REAL unconditional slots, then
    COND_SLOTS conditional slots, then an unconditional tail. A core with
    nact active chunks skips the FIRST (CAP-nact) conditional slots
    (cond DMAs with bounds_check=skip_entire_dma: no data moved, semaphore
    still bumped), so the static DVE/sem pipeline is unchanged. The STT for
    a skipped slot reads garbage SBUF into an acc column the host ignores —
    and because skipped slots sit near the FRONT of the schedule, those
    stale STTs run during early-stream DVE slack instead of serializing
    after the last real chunk (which is always unconditional). nact is a
    per-core uint32 input pulled into a Sync register (~1us HBM ldr) between
    the slot-0 and slot-1 DMA issues, well before its first use.
  - Per chunk one fused DVE scalar_tensor_tensor (elementwise mult + free-
    axis add-reduce via accum_out; product discarded through a stride-0
    broadcast output) accumulates into acc[:, j]. Host combines the
    8 x [128, n_i] partials in float64.

Raw hand-rolled semaphores (no TileContext): avoids its ~7us preamble +
~10us epilogue. The NRT-injected postamble (all-sem zeroing, ~7us) and the
const-AP preamble (~2us) are fixed costs outside kernel control.
"""

import os

import numpy as np

import concourse.bacc as bacc
import concourse.mybir as mybir
from concourse.bass_utils import run_bass_kernel_spmd

D, N = 512, 65536
NCORES = 8
P = 128                          # SBUF partitions
TOTAL_ELEMS = D * N              # 2^25 per tensor

F = int(os.environ.get("COSLOSS_FCHUNK", "1024"))    # cols per chunk
CAP = int(os.environ.get("COSLOSS_CAP", "36"))       # chunk slots in the NEFF
HEAD_REAL = int(os.environ.get("COSLOSS_HEAD", "4"))  # unconditional head slots
COND_SLOTS = int(os.environ.get("COSLOSS_COND", "12"))  # conditional slots
MINCH = CAP - COND_SLOTS                             # minimum active chunks
SLOTS = int(os.environ.get("COSLOSS_SLOTS", "16"))   # SBUF ring slots
CHUNK_ELEMS = P * F                                  # 131072 for F=1024
GLOBAL_CHUNKS = TOTAL_ELEMS // CHUNK_ELEMS           # 256 for F=1024

# Per-device relative bandwidth weights (jax device order). Measured per-core
# HBM GB/s under balanced load over repeated runs on this 8-core terminal:
# device 6 (physical nc0) is stably slow (~312-320 GB/s), the rest sit near
# 360-380 with sizable run-to-run wobble, so these are robust averages.
_DEFAULT_WEIGHTS = (370.0, 375.0, 375.0, 377.0, 368.0, 370.0, 312.0, 378.0)

IMPL = os.environ.get("COSLOSS_IMPL", "bal")

_CACHE = {}


def _chunk_alloc(weights=None):
    """Largest-remainder apportionment of GLOBAL_CHUNKS to 8 cores."""
    if weights is None:
        env = os.environ.get("COSLOSS_N")
        if env:
            n = [int(x) for x in env.split(",")]
            assert len(n) == NCORES and sum(n) == GLOBAL_CHUNKS, n
            return n
        wenv = os.environ.get("COSLOSS_WEIGHTS")
        weights = (
            [float(x) for x in wenv.split(",")] if wenv else list(_DEFAULT_WEIGHTS)
        )
    w = np.asarray(weights, dtype=np.float64)
    exact = GLOBAL_CHUNKS * w / w.sum()
    n = np.floor(exact).astype(int)
    rem = exact - n
    for i in np.argsort(-rem)[: GLOBAL_CHUNKS - n.sum()]:
        n[i] += 1
    n = np.clip(n, MINCH + 1, CAP)
    # rebalance if clipping broke the sum (shift to/from the largest slots)
    while n.sum() != GLOBAL_CHUNKS:
        if n.sum() < GLOBAL_CHUNKS:
            i = np.argmin(n / w)
            assert n[i] < CAP
            n[i] += 1
        else:
            i = np.argmax(n / w)
            assert n[i] > MINCH + 1
            n[i] -= 1
    assert n.sum() == GLOBAL_CHUNKS and (n > MINCH).all() and (n <= CAP).all(), n
    return [int(x) for x in n]


def _build_balanced():
    import contextlib

    import concourse.bass as bass  # noqa: F401  (engine types via mybir)

    nc = bacc.Bacc(None)
    ncols = CAP * F
    f_in = nc.declare_dram_parameter("feats", [P, ncols], mybir.dt.float32, isOutput=False)
    w_in = nc.declare_dram_parameter("warped", [P, ncols], mybir.dt.float32, isOutput=False)
    nact_in = nc.declare_dram_parameter("nact", [1, 1], mybir.dt.uint32, isOutput=False)
    # CAP+1 acc/out cols: col CAP holds the second half-STT of the last slot.
    out = nc.declare_dram_parameter(
        "partial", [P, CAP + 1], mybir.dt.float32, isOutput=True
    )

    head = CAP - 4  # acc cols written out early vs at the end
    assert SLOTS <= MINCH and 2 * SLOTS * F * 4 <= 176 * 1024, (SLOTS, F)
    assert 2 <= HEAD_REAL and HEAD_REAL + COND_SLOTS <= CAP

    with (
        nc.sbuf_tensor([P, SLOTS * F], mybir.dt.float32) as ftile,
        nc.sbuf_tensor([P, SLOTS * F], mybir.dt.float32) as wtile,
        nc.sbuf_tensor([P, CAP + 1], mybir.dt.float32) as acc,
        nc.sbuf_tensor([P, 1], mybir.dt.float32) as dummy,
    ):
        with contextlib.ExitStack() as ctx:
            dsems = [ctx.enter_context(nc.semaphore(f"dsem{j}")) for j in range(CAP)]
            vsem = ctx.enter_context(nc.semaphore("vsem"))
            osem = ctx.enter_context(nc.semaphore("osem"))
            nact_reg = ctx.enter_context(nc.sync.register("nact_reg"))
            sem_nums = sorted(s.num for s in [*dsems, vsem, osem])
            assert sem_nums == list(
                range(sem_nums[0], sem_nums[0] + len(sem_nums))
            ), sem_nums
            sem_range = range(sem_nums[0], sem_nums[-1] + 1)

            with nc.Block(no_gpsimd_drain=True) as block:

                @block.sync
                def _(sync):
                    nact = None
                    for j in range(CAP):
                        if j == 1:
                            # Load this core's active-chunk count while the
                            # slot-0 data drains; first used at slot
                            # HEAD_REAL, long after the ~1us HBM ldr lands.
                            sync.reg_load(nact_reg, nact_in[0:1, 0:1])
                            nact = sync.snap(nact_reg, min_val=0, max_val=CAP)
                        s = j % SLOTS
                        if j >= SLOTS:
                            # WAR: slot s is being read by STT_{j-SLOTS};
                            # HWDGE issue is FIFO per ring, so this wait
                            # also orders this ring's later DMAs behind it.
                            sync.wait_ge(vsem, j - SLOTS + 1)
                        in_cond_block = HEAD_REAL <= j < HEAD_REAL + COND_SLOTS
                        # slot j in the cond block is skipped iff
                        # j - HEAD_REAL < CAP - nact  (first CAP-nact slots)
                        kw = (
                            {"cond": nact > CAP - j + HEAD_REAL - 1}
                            if in_cond_block
                            else {}
                        )
                        sync.dma_start(
                            ftile[:, s * F : (s + 1) * F],
                            f_in[:, j * F : (j + 1) * F],
                            **kw,
                        ).then_inc(dsems[j], 16)
                        sync.dma_start(
                            wtile[:, s * F : (s + 1) * F],
                            w_in[:, j * F : (j + 1) * F],
                            **kw,
                        ).then_inc(dsems[j], 16)
                    # Write out the bulk of acc early (overlaps the tail of
                    # the input stream); only the last columns stay on the
                    # post-stream critical path.
                    sync.wait_ge(vsem, head)
                    sync.dma_start(out[:, :head], acc[:, :head]).then_inc(osem, 16)
                    sync.wait_ge(vsem, CAP + 1)
                    sync.dma_start(out[:, head:], acc[:, head:]).then_inc(osem, 16)

                @block.vector
                def _(vector):
                    for j in range(CAP):
                        s = j % SLOTS
                        vector.wait_ge(dsems[j], 32)
                        # out = (ft * 1.0) * wt (discarded via stride-0
                        # broadcast), accum_out = per-partition sum. For a
                        # cond-skipped slot this reads garbage SBUF into an
                        # acc column the host masks out. The last slot is
                        # processed as two half-STTs (summed by the host) so
                        # only ~F/2 cols of DVE work trail the final DMA.
                        pieces = (
                            [(0, F)]
                            if j < CAP - 1
                            else [(0, F // 2), (F // 2, F - F // 2)]
                        )
                        for pi, (poff, psz) in enumerate(pieces):
                            col = j if pi == 0 else CAP
                            lo = s * F + poff
                            nc.vector.scalar_tensor_tensor(
                                dummy[:, :].broadcast_to((P, psz)),
                                ftile[:, lo : lo + psz],
                                1.0,
                                wtile[:, lo : lo + psz],
                                op0=mybir.AluOpType.mult,
                                op1=mybir.AluOpType.mult,
                                accum_out=acc[:, col : col + 1],
                            ).then_inc(vsem, 1)

                @block.gpsimd
                def _(gpsimd):
                    # osem at its final value implies both out-DMAs landed,
                    # which implies every earlier sem reached its final
                    # value. Reset them so the NEFF is safe to re-execute.
                    gpsimd.wait_ge(osem, 32)
                    gpsimd.dma_reset(sem_range)
                    gpsimd.sem_clear(sem_range)

    nc.finalize()
    return nc


def _build_raw_even():
    """Previous even-shard builder (64 rows/core, F=2048) kept as fallback."""
    nc = bacc.Bacc(None)
    import contextlib

    F0, M0, SLOTS0 = 2048, 32768, 8
    nch = M0 // F0  # 16
    chunks = []
    for j in range(nch):
        if j == nch - 1:
            q = F0 // 4
            for k in range(4):
                chunks.append((j * F0 + k * q, q))
        else:
            chunks.append((j * F0, F0))
    nchunks = len(chunks)
    head = nchunks - 4
    f_in = nc.declare_dram_parameter("feats", [P, M0], mybir.dt.float32, isOutput=False)
    w_in = nc.declare_dram_parameter("warped", [P, M0], mybir.dt.float32, isOutput=False)
    out = nc.declare_dram_parameter("partial", [P, nchunks], mybir.dt.float32, isOutput=True)

    with (
        nc.sbuf_tensor([P, SLOTS0 * F0], mybir.dt.float32) as ftile,
        nc.sbuf_tensor([P, SLOTS0 * F0], mybir.dt.float32) as wtile,
        nc.sbuf_tensor([P, nchunks], mybir.dt.float32) as acc,
        nc.sbuf_tensor([P, 1], mybir.dt.float32) as dummy,
    ):
        with contextlib.ExitStack() as ctx:
            dsems = [ctx.enter_context(nc.semaphore(f"dsem{j}")) for j in range(nchunks)]
            vsem = ctx.enter_context(nc.semaphore("vsem"))
            osem = ctx.enter_context(nc.semaphore("osem"))
            sem_nums = sorted(s.num for s in [*dsems, vsem, osem])
            assert sem_nums == list(range(sem_nums[0], sem_nums[0] + len(sem_nums)))
            sem_range = range(sem_nums[0], sem_nums[-1] + 1)

            with nc.Block(no_gpsimd_drain=True) as block:

                @block.sync
                def _(sync):
                    for j, (off, sz) in enumerate(chunks):
                        s = j % SLOTS0
                        if j >= SLOTS0:
                            sync.wait_ge(vsem, j - SLOTS0 + 1)
                        sync.dma_start(
                            ftile[:, s * F0 : s * F0 + sz], f_in[:, off : off + sz]
                        ).then_inc(dsems[j], 16)
                        sync.dma_start(
                            wtile[:, s * F0 : s * F0 + sz], w_in[:, off : off + sz]
                        ).then_inc(dsems[j], 16)
                    sync.wait_ge(vsem, head)
                    sync.dma_start(out[:, :head], acc[:, :head]).then_inc(osem, 16)
                    sync.wait_ge(vsem, nchunks)
                    sync.dma_start(out[:, head:], acc[:, head:]).then_inc(osem, 16)

                @block.vector
                def _(vector):
                    for j, (off, sz) in enumerate(chunks):
                        s = j % SLOTS0
                        vector.wait_ge(dsems[j], 32)
                        nc.vector.scalar_tensor_tensor(
                            dummy[:, :].broadcast_to((P, sz)),
                            ftile[:, s * F0 : s * F0 + sz],
                            1.0,
                            wtile[:, s * F0 : s * F0 + sz],
                            op0=mybir.AluOpType.mult,
                            op1=mybir.AluOpType.mult,
                            accum_out=acc[:, j : j + 1],
                        ).then_inc(vsem, 1)

                @block.gpsimd
                def _(gpsimd):
                    gpsimd.wait_ge(osem, 32)
                    gpsimd.dma_reset(sem_range)
                    gpsimd.sem_clear(sem_range)

    nc.finalize()
    return nc


def _get_nc(impl=None):
    impl = impl or IMPL
    if impl not in _CACHE:
        _CACHE[impl] = _build_balanced() if impl == "bal" else _build_raw_even()
    return _CACHE[impl]


def _active_slots(n_chunks):
    m = CAP - n_chunks  # skipped = first m slots of the cond block
    assert 0 <= m <= COND_SLOTS, n_chunks
    return list(range(HEAD_REAL)) + list(range(HEAD_REAL + m, CAP))


def _active_cols(n_chunks):
    # acc col CAP is the second half of (always-real) slot CAP-1.
    return _active_slots(n_chunks) + [CAP]


def _pack(flat, start_chunk, n_chunks):
    buf = np.zeros((P, CAP * F), dtype=np.float32)
    sl = flat[
        start_chunk * CHUNK_ELEMS : (start_chunk + n_chunks) * CHUNK_ELEMS
    ].reshape(n_chunks, P, F)
    for i, slot in enumerate(_active_slots(n_chunks)):
        buf[:, slot * F : (slot + 1) * F] = sl[i]
    return buf


def _run(feats, warped_feats, impl=None, **spmd_kwargs):
    feats = np.ascontiguousarray(np.asarray(feats), dtype=np.float32)
    warped = np.ascontiguousarray(np.asarray(warped_feats), dtype=np.float32)
    assert feats.shape == (D, N) and warped.shape == (D, N)
    impl = impl or IMPL

    if impl == "bal":
        n = _chunk_alloc()
        ff, wf = feats.reshape(-1), warped.reshape(-1)
        starts = np.concatenate([[0], np.cumsum(n)])
        in_maps = [
            {
                "feats": _pack(ff, starts[c], n[c]),
                "warped": _pack(wf, starts[c], n[c]),
                "nact": np.array([[n[c]]], dtype=np.uint32),
            }
            for c in range(NCORES)
        ]
    else:
        n = None
        DSHARD, M0 = D // NCORES, 32768
        in_maps = [
            {
                "feats": feats[c * DSHARD : (c + 1) * DSHARD].reshape(P, M0),
                "warped": warped[c * DSHARD : (c + 1) * DSHARD].reshape(P, M0),
            }
            for c in range(NCORES)
        ]
    res = run_bass_kernel_spmd(
        _get_nc(impl), in_maps, core_ids=list(range(NCORES)), **spmd_kwargs
    )
    res.chunk_alloc = n
    return res


def gather_partials(res):
    """Mask-aware reduction of per-core partials to the scalar loss."""
    n = getattr(res, "chunk_alloc", None)
    total = 0.0
    for c, r in enumerate(res.results):
        p = r["partial"].astype(np.float64)
        if n is not None:
            p = p[:, _active_cols(n[c])]
        total += float(p.sum())
    return np.array(1.0 - total / N, dtype=np.float32)


def kernel(feats, warped_feats):
    return gather_partials(_run(feats, warped_feats))


# revision 19
# speedup vs baseline: 1.1302x; 1.1117x over previous
"""CosineDistanceLoss kernel for Trainium2 (8 NeuronCores, Bass).

reference: mean_n(1 - sum_d feats[d,n] * warped_feats[d,n])
         = 1 - (1/N) * sum_{d,n} feats[d,n] * warped_feats[d,n]

The loss is a single global sum of the elementwise product, so ANY disjoint
partition of elements across cores is valid. The kernel is pure HBM streaming
(64 MiB/stack total; DVE has ~3x slack), and the measured per-core HBM
bandwidth is ASYMMETRIC and partly stable (nc0 sustains ~320 GB/s while its
stack partner nc1 gets ~401 GB/s; other cores land in 335-402). Since the
graded time is the MAX over cores, we balance: each core gets a slice of the
global element stream sized proportionally to its measured bandwidth.

Mechanics (one NEFF for all cores, shapes must match):
  - The global 2^25-element stream per tensor is cut into 256 chunks of
    128x1024 (0.5 MiB). Core i takes n_i consecutive chunks (sum n_i = 256),
    packed by the host into a [128, CAP*1024] DRAM buffer (first n_i*1024
    cols are real data, rest never read).
  - The kernel has CAP=36 chunk slots: HEAD_REAL unconditional slots, then
    COND_SLOTS conditional slots, then an unconditional tail. A core with
    nact active chunks skips the FIRST (CAP-nact) conditional slots
    (cond DMAs with bounds_check=skip_entire_dma: no data moved, semaphore
    still bumped), so the static DVE/sem pipeline is unchanged. The STT for
    a skipped slot reads garbage SBUF into an acc column the host ignores —
    and because skipped slots sit near the FRONT of the schedule, those
    stale STTs run during early-stream DVE slack instead of serializing
    after the last real chunk (which is always unconditional). nact is a
    per-core uint32 input pulled into a Sync register (~1us HBM ldr) between
    the slot-0 and slot-1 DMA issues, well before its first use.
  - Per chunk one fused DVE scalar_tensor_tensor (elementwise mult + free-
    axis add-reduce via accum_out; product discarded through a stride-0
    broadcast output) accumulates into acc[:, j]. Host combines the
    8 x [128, n_i] partials in float64.

Raw hand-rolled semaphores (no TileContext): avoids its ~7us preamble +
~10us epilogue. The NRT-injected postamble (all-sem zeroing, ~7us) and the
const-AP preamble (~2us) are fixed costs outside kernel control.
"""

import os

import numpy as np

import concourse.bacc as bacc
import concourse.mybir as mybir
from concourse.bass_utils import run_bass_kernel_spmd

D, N = 512, 65536
NCORES = 8
P = 128                          # SBUF partitions
TOTAL_ELEMS = D * N              # 2^25 per tensor

F = int(os.environ.get("COSLOSS_FCHUNK", "1024"))    # cols per chunk
CAP = int(os.environ.get("COSLOSS_CAP", "36"))       # chunk slots in the NEFF
HEAD_REAL = int(os.environ.get("COSLOSS_HEAD", "4"))  # unconditional head slots
COND_SLOTS = int(os.environ.get("COSLOSS_COND", "12"))  # conditional slots
MINCH = CAP - COND_SLOTS                             # minimum active chunks
SLOTS = int(os.environ.get("COSLOSS_SLOTS", "16"))   # SBUF ring slots
CHUNK_ELEMS = P * F                                  # 131072 for F=1024
GLOBAL_CHUNKS = TOTAL_ELEMS // CHUNK_ELEMS           # 256 for F=1024

# Per-device relative bandwidth weights (jax device order). Measured per-core
# HBM GB/s under balanced load over repeated runs on this 8-core terminal:
# device 6 (physical nc0) is stably slow (~312-320 GB/s), the rest sit near
# 360-380 with sizable run-to-run wobble, so these are robust averages.
_DEFAULT_WEIGHTS = (370.0, 375.0, 375.0, 377.0, 368.0, 370.0, 312.0, 378.0)

IMPL = os.environ.get("COSLOSS_IMPL", "bal")

_CACHE = {}


def _chunk_alloc(weights=None):
    """Largest-remainder apportionment of GLOBAL_CHUNKS to 8 cores."""
    if weights is None:
        env = os.environ.get("COSLOSS_N")
        if env:
            n = [int(x) for x in env.split(",")]
            assert len(n) == NCORES and sum(n) == GLOBAL_CHUNKS, n
            return n
        wenv = os.environ.get("COSLOSS_WEIGHTS")
        weights = (
            [float(x) for x in wenv.split(",")] if wenv else list(_DEFAULT_WEIGHTS)
        )
    w = np.asarray(weights, dtype=np.float64)
    exact = GLOBAL_CHUNKS * w / w.sum()
    n = np.floor(exact).astype(int)
    rem = exact - n
    for i in np.argsort(-rem)[: GLOBAL_CHUNKS - n.sum()]:
        n[i] += 1
    n = np.clip(n, MINCH + 1, CAP)
    # rebalance if clipping broke the sum (shift to/from the largest slots)
    while n.sum() != GLOBAL_CHUNKS:
        if n.sum() < GLOBAL_CHUNKS:
            i = np.argmin(n / w)
            assert n[i] < CAP
            n[i] += 1
        else:
            i = np.argmax(n / w)
            assert n[i] > MINCH + 1
            n[i] -= 1
    assert n.sum() == GLOBAL_CHUNKS and (n > MINCH).all() and (n <= CAP).all(), n
    return [int(x) for x in n]


def _build_balanced():
    import contextlib

    import concourse.bass as bass  # noqa: F401  (engine types via mybir)

    nc = bacc.Bacc(None)
    ncols = CAP * F
    f_in = nc.declare_dram_parameter("feats", [P, ncols], mybir.dt.float32, isOutput=False)
    w_in = nc.declare_dram_parameter("warped", [P, ncols], mybir.dt.float32, isOutput=False)
    nact_in = nc.declare_dram_parameter("nact", [1, 1], mybir.dt.uint32, isOutput=False)
    # CAP+1 acc/out cols: col CAP holds the second half-STT of the last slot.
    out = nc.declare_dram_parameter(
        "partial", [P, CAP + 1], mybir.dt.float32, isOutput=True
    )

    head = CAP - 4  # acc cols written out early vs at the end
    assert SLOTS <= MINCH and 2 * SLOTS * F * 4 <= 176 * 1024, (SLOTS, F)
    assert 2 <= HEAD_REAL and HEAD_REAL + COND_SLOTS <= CAP

    with (
        nc.sbuf_tensor([P, SLOTS * F], mybir.dt.float32) as ftile,
        nc.sbuf_tensor([P, SLOTS * F], mybir.dt.float32) as wtile,
        nc.sbuf_tensor([P, CAP + 1], mybir.dt.float32) as acc,
        nc.sbuf_tensor([P, 1], mybir.dt.float32) as dummy,
    ):
        with contextlib.ExitStack() as ctx:
            # dsems[CAP] guards the second half of the last slot's data (the
            # last slot is streamed as two half-DMAs so the first half-STT
            # overlaps the second half's drain).
            dsems = [
                ctx.enter_context(nc.semaphore(f"dsem{j}")) for j in range(CAP + 1)
            ]
            vsem = ctx.enter_context(nc.semaphore("vsem"))
            osem = ctx.enter_context(nc.semaphore("osem"))
            nact_reg = ctx.enter_context(nc.sync.register("nact_reg"))
            sem_nums = sorted(s.num for s in [*dsems, vsem, osem])
            assert sem_nums == list(
                range(sem_nums[0], sem_nums[0] + len(sem_nums))
            ), sem_nums
            sem_range = range(sem_nums[0], sem_nums[-1] + 1)

            with nc.Block(no_gpsimd_drain=True) as block:

                @block.sync
                def _(sync):
                    nact = None
                    for j in range(CAP):
                        if j == 1:
                            # Load this core's active-chunk count while the
                            # slot-0 data drains; first used at slot
                            # HEAD_REAL, long after the ~1us HBM ldr lands.
                            sync.reg_load(nact_reg, nact_in[0:1, 0:1])
                            nact = sync.snap(nact_reg, min_val=0, max_val=CAP)
                        s = j % SLOTS
                        if j >= SLOTS:
                            # WAR: slot s is being read by STT_{j-SLOTS};
                            # HWDGE issue is FIFO per ring, so this wait
                            # also orders this ring's later DMAs behind it.
                            sync.wait_ge(vsem, j - SLOTS + 1)
                        in_cond_block = HEAD_REAL <= j < HEAD_REAL + COND_SLOTS
                        # slot j in the cond block is skipped iff
                        # j - HEAD_REAL < CAP - nact  (first CAP-nact slots)
                        kw = (
                            {"cond": nact > CAP - j + HEAD_REAL - 1}
                            if in_cond_block
                            else {}
                        )
                        if j < CAP - 1:
                            halves = [(0, F, dsems[j])]
                        else:
                            h = F // 2
                            halves = [(0, h, dsems[j]), (h, F - h, dsems[CAP])]
                        for hoff, hsz, hsem in halves:
                            sync.dma_start(
                                ftile[:, s * F + hoff : s * F + hoff + hsz],
                                f_in[:, j * F + hoff : j * F + hoff + hsz],
                                **kw,
                            ).then_inc(hsem, 16)
                            sync.dma_start(
                                wtile[:, s * F + hoff : s * F + hoff + hsz],
                                w_in[:, j * F + hoff : j * F + hoff + hsz],
                                **kw,
                            ).then_inc(hsem, 16)
                    # Write out the bulk of acc early (overlaps the tail of
                    # the input stream); only the last columns stay on the
                    # post-stream critical path.
                    sync.wait_ge(vsem, head)
                    sync.dma_start(out[:, :head], acc[:, :head]).then_inc(osem, 16)
                    sync.wait_ge(vsem, CAP + 1)
                    sync.dma_start(out[:, head:], acc[:, head:]).then_inc(osem, 16)

                @block.vector
                def _(vector):
                    for j in range(CAP):
                        s = j % SLOTS
                        # out = (ft * 1.0) * wt (discarded via stride-0
                        # broadcast), accum_out = per-partition sum. For a
                        # cond-skipped slot this reads garbage SBUF into an
                        # acc column the host masks out. The last slot is
                        # two half-STTs on the two half-DMAs (summed by the
                        # host) so only ~F/2 cols of DVE work trail the
                        # final DMA.
                        if j < CAP - 1:
                            pieces = [(0, F, dsems[j], j)]
                        else:
                            h = F // 2
                            pieces = [
                                (0, h, dsems[j], j),
                                (h, F - h, dsems[CAP], CAP),
                            ]
                        for poff, psz, psem, col in pieces:
                            vector.wait_ge(psem, 32)
                            lo = s * F + poff
                            nc.vector.scalar_tensor_tensor(
                                dummy[:, :].broadcast_to((P, psz)),
                                ftile[:, lo : lo + psz],
                                1.0,
                                wtile[:, lo : lo + psz],
                                op0=mybir.AluOpType.mult,
                                op1=mybir.AluOpType.mult,
                                accum_out=acc[:, col : col + 1],
                            ).then_inc(vsem, 1)

                @block.gpsimd
                def _(gpsimd):
                    # osem at its final value implies both out-DMAs landed,
                    # which implies every earlier sem reached its final
                    # value. Reset them so the NEFF is safe to re-execute.
                    gpsimd.wait_ge(osem, 32)
                    gpsimd.dma_reset(sem_range)
                    gpsimd.sem_clear(sem_range)

    nc.finalize()
    return nc


def _build_raw_even():
    """Previous even-shard builder (64 rows/core, F=2048) kept as fallback."""
    nc = bacc.Bacc(None)
    import contextlib

    F0, M0, SLOTS0 = 2048, 32768, 8
    nch = M0 // F0  # 16
    chunks = []
    for j in range(nch):
        if j == nch - 1:
            q = F0 // 4
            for k in range(4):
                chunks.append((j * F0 + k * q, q))
        else:
            chunks.append((j * F0, F0))
    nchunks = len(chunks)
    head = nchunks - 4
    f_in = nc.declare_dram_parameter("feats", [P, M0], mybir.dt.float32, isOutput=False)
    w_in = nc.declare_dram_parameter("warped", [P, M0], mybir.dt.float32, isOutput=False)
    out = nc.declare_dram_parameter("partial", [P, nchunks], mybir.dt.float32, isOutput=True)

    with (
        nc.sbuf_tensor([P, SLOTS0 * F0], mybir.dt.float32) as ftile,
        nc.sbuf_tensor([P, SLOTS0 * F0], mybir.dt.float32) as wtile,
        nc.sbuf_tensor([P, nchunks], mybir.dt.float32) as acc,
        nc.sbuf_tensor([P, 1], mybir.dt.float32) as dummy,
    ):
        with contextlib.ExitStack() as ctx:
            dsems = [ctx.enter_context(nc.semaphore(f"dsem{j}")) for j in range(nchunks)]
            vsem = ctx.enter_context(nc.semaphore("vsem"))
            osem = ctx.enter_context(nc.semaphore("osem"))
            sem_nums = sorted(s.num for s in [*dsems, vsem, osem])
            assert sem_nums == list(range(sem_nums[0], sem_nums[0] + len(sem_nums)))
            sem_range = range(sem_nums[0], sem_nums[-1] + 1)

            with nc.Block(no_gpsimd_drain=True) as block:

                @block.sync
                def _(sync):
                    for j, (off, sz) in enumerate(chunks):
                        s = j % SLOTS0
                        if j >= SLOTS0:
                            sync.wait_ge(vsem, j - SLOTS0 + 1)
                        sync.dma_start(
                            ftile[:, s * F0 : s * F0 + sz], f_in[:, off : off + sz]
                        ).then_inc(dsems[j], 16)
                        sync.dma_start(
                            wtile[:, s * F0 : s * F0 + sz], w_in[:, off : off + sz]
                        ).then_inc(dsems[j], 16)
                    sync.wait_ge(vsem, head)
                    sync.dma_start(out[:, :head], acc[:, :head]).then_inc(osem, 16)
                    sync.wait_ge(vsem, nchunks)
                    sync.dma_start(out[:, head:], acc[:, head:]).then_inc(osem, 16)

                @block.vector
                def _(vector):
                    for j, (off, sz) in enumerate(chunks):
                        s = j % SLOTS0
                        vector.wait_ge(dsems[j], 32)
                        nc.vector.scalar_tensor_tensor(
                            dummy[:, :].broadcast_to((P, sz)),
                            ftile[:, s * F0 : s * F0 + sz],
                            1.0,
                            wtile[:, s * F0 : s * F0 + sz],
                            op0=mybir.AluOpType.mult,
                            op1=mybir.AluOpType.mult,
                            accum_out=acc[:, j : j + 1],
                        ).then_inc(vsem, 1)

                @block.gpsimd
                def _(gpsimd):
                    gpsimd.wait_ge(osem, 32)
                    gpsimd.dma_reset(sem_range)
                    gpsimd.sem_clear(sem_range)

    nc.finalize()
    return nc


def _get_nc(impl=None):
    impl = impl or IMPL
    if impl not in _CACHE:
        _CACHE[impl] = _build_balanced() if impl == "bal" else _build_raw_even()
    return _CACHE[impl]


def _active_slots(n_chunks):
    m = CAP - n_chunks  # skipped = first m slots of the cond block
    assert 0 <= m <= COND_SLOTS, n_chunks
    return list(range(HEAD_REAL)) + list(range(HEAD_REAL + m, CAP))


def _active_cols(n_chunks):
    # acc col CAP is the second half of (always-real) slot CAP-1.
    return _active_slots(n_chunks) + [CAP]


def _pack(flat, start_chunk, n_chunks):
    buf = np.zeros((P, CAP * F), dtype=np.float32)
    sl = flat[
        start_chunk * CHUNK_ELEMS : (start_chunk + n_chunks) * CHUNK_ELEMS
    ].reshape(n_chunks, P, F)
    for i, slot in enumerate(_active_slots(n_chunks)):
        buf[:, slot * F : (slot + 1) * F] = sl[i]
    return buf


def _run(feats, warped_feats, impl=None, **spmd_kwargs):
    feats = np.ascontiguousarray(np.asarray(feats), dtype=np.float32)
    warped = np.ascontiguousarray(np.asarray(warped_feats), dtype=np.float32)
    assert feats.shape == (D, N) and warped.shape == (D, N)
    impl = impl or IMPL

    if impl == "bal":
        n = _chunk_alloc()
        ff, wf = feats.reshape(-1), warped.reshape(-1)
        starts = np.concatenate([[0], np.cumsum(n)])
        in_maps = [
            {
                "feats": _pack(ff, starts[c], n[c]),
                "warped": _pack(wf, starts[c], n[c]),
                "nact": np.array([[n[c]]], dtype=np.uint32),
            }
            for c in range(NCORES)
        ]
    else:
        n = None
        DSHARD, M0 = D // NCORES, 32768
        in_maps = [
            {
                "feats": feats[c * DSHARD : (c + 1) * DSHARD].reshape(P, M0),
                "warped": warped[c * DSHARD : (c + 1) * DSHARD].reshape(P, M0),
            }
            for c in range(NCORES)
        ]
    res = run_bass_kernel_spmd(
        _get_nc(impl), in_maps, core_ids=list(range(NCORES)), **spmd_kwargs
    )
    res.chunk_alloc = n
    return res


def gather_partials(res):
    """Mask-aware reduction of per-core partials to the scalar loss."""
    n = getattr(res, "chunk_alloc", None)
    total = 0.0
    for c, r in enumerate(res.results):
        p = r["partial"].astype(np.float64)
        if n is not None:
            p = p[:, _active_cols(n[c])]
        total += float(p.sum())
    return np.array(1.0 - total / N, dtype=np.float32)


def kernel(feats, warped_feats):
    return gather_partials(_run(feats, warped_feats))
